# revision 10
# baseline (speedup 1.0000x reference)
"""Trainium2 Bass kernel for nn_AdversMaskEdge (gnn_message_passing).

Computation (per edge e): gather h[l, src[e]], h[l, dst[e]] (l=0,1, D=128);
cross features x = concat_{i,j} (src_i * dst_j)  [512]; x = relu(x @ W0.T + b0);
pos = x @ W1.T + b1; logits = pos @ Wf.T + bf; z = logits + gumbel(u);
output = one_hot(argmax(z), 2)  (straight-through value == y_hard exactly).

Strategy:
  - Shard E=160000 edges across 8 cores (20000 each, padded to 20096 = 128*157).
  - Edge -> (partition p, chunk c) mapping: e_local = p*157 + c, so per-partition
    rows are contiguous for the u-load and the output store.
  - h is reshaped host-side to [N, L*D] fp16 rows (512B) and fetched with
    gpsimd.dma_gather (SWDGE indexed gather, 512B descriptors).
  - Gathered [e,d] tiles are transposed to [d,e] on the PE (fp16, 1 cyc/col).
  - Cross products on DVE in one big f32 tensor_tensor per supertile.
  - MLP matmuls run in float32r (TF32); W1/Wf are folded host-side (Weff=Wf@W1).
  - Gumbel + compare produce the one-hot in edge-partition layout; margins are
    also written out.
  - Low-precision pass is refined host-side (f64) for the ~300 edges with
    |margin| < TAU; rounding noise is ~4e-4 so TAU=4e-3 is a >8-sigma net.
"""

import numpy as np

import concourse.bass as bass
import concourse.bacc as bacc
import concourse.mybir as mybir
import concourse.tile as tile
from concourse.bass_utils import run_bass_kernel_spmd

# Problem constants (hardcoded per harness contract)
L, N, D, E = 2, 10000, 128, 160000
EPS = 1e-10
NCORES = 8
E_PER = E // NCORES            # 20000
CH = 157                        # chunks of 128 edges per core
EPAD = 128 * CH                 # 20096
SLAB_CH = 8                     # chunks gathered per dma_gather (1024 idxs; 2048 overflows the SWDGE ring)
NCH_ST = 2                      # chunks per compute supertile
TAU = 4e-3                      # |margin| refinement threshold

f32 = mybir.dt.float32
f32r = mybir.dt.float32r
f16 = mybir.dt.float16
i16 = mybir.dt.int16
AF = mybir.ActivationFunctionType
ALU = mybir.AluOpType


def build_program(ch=CH, slab_ch=SLAB_CH, nch_st=NCH_ST):
    CHL, SLABL, NCHL = ch, slab_ch, nch_st
    nc = bacc.Bacc(trn_type="TRN2")

    h16 = nc.dram_tensor("h16", [N, L * D], f16, kind="ExternalInput")
    w0t = nc.dram_tensor("w0t", [D, 4 * D], f32r, kind="ExternalInput")
    wefft = nc.dram_tensor("wefft", [D, 2], f32r, kind="ExternalInput")
    b0d = nc.dram_tensor("b0d", [D, 1], f32, kind="ExternalInput")
    ident = nc.dram_tensor("ident", [D, D], f16, kind="ExternalInput")
    isrc = nc.dram_tensor("isrc", [128, CHL * 8], i16, kind="ExternalInput")
    idst = nc.dram_tensor("idst", [128, CHL * 8], i16, kind="ExternalInput")
    ud = nc.dram_tensor("ud", [128, CHL * 2 + 1], f32, kind="ExternalInput")
    outd = nc.dram_tensor("outd", [128 * CHL, 2], f32, kind="ExternalOutput")
    margd = nc.dram_tensor("margd", [128 * CHL], f32, kind="ExternalOutput")

    with tile.TileContext(nc) as tc:
        with (
            tc.tile_pool(name="const", bufs=1) as cpool,
            tc.tile_pool(name="gath", bufs=2) as gpool,
            tc.tile_pool(name="work", bufs=2) as wpool,
            tc.tile_pool(name="psT", bufs=2, space="PSUM") as ppool,
            tc.tile_pool(name="fin", bufs=1) as fpool,
        ):
            # ---- preamble loads ----
            w0t_sb = cpool.tile([D, 4 * D], f32r, tag="w0t")
            nc.sync.dma_start(w0t_sb[:], w0t[:, :])
            wefft_sb = cpool.tile([D, 2], f32r, tag="wefft")
            nc.sync.dma_start(wefft_sb[:], wefft[:, :])
            b0_sb = cpool.tile([D, 1], f32, tag="b0")
            nc.sync.dma_start(b0_sb[:], b0d[:, :])
            id_sb = cpool.tile([D, D], f16, tag="ident")
            nc.sync.dma_start(id_sb[:], ident[:, :])
            isrc_sb = cpool.tile([128, CHL * 8], i16, tag="isrc")
            nc.sync.dma_start(isrc_sb[:], isrc[:, :])
            idst_sb = cpool.tile([128, CHL * 8], i16, tag="idst")
            nc.sync.dma_start(idst_sb[:], idst[:, :])
            u_sb = fpool.tile([128, CHL * 2 + 1], f32, tag="u")
            nc.sync.dma_start(u_sb[:], ud[:, :])

            logits_sb = fpool.tile([128, CHL * 2], f32, tag="logits")

            # ---- main loop: slabs of SLAB_CH chunks ----
            n_slabs = (CHL + SLABL - 1) // SLABL
            for b in range(n_slabs):
                ch0 = b * SLABL
                nch_slab = min(SLABL, CHL - ch0)
                nidx = nch_slab * 128
                gsrc = gpool.tile([128, nch_slab, L * D], f16, tag="gsrc")
                gdst = gpool.tile([128, nch_slab, L * D], f16, tag="gdst")
                icol = ch0 * 8  # 128 idx/chunk / 16 per col
                nc.gpsimd.dma_gather(
                    gsrc[:], h16[:, :], isrc_sb[:, icol : icol + nidx // 16],
                    nidx, nidx, L * D,
                )
                nc.gpsimd.dma_gather(
                    gdst[:], h16[:, :], idst_sb[:, icol : icol + nidx // 16],
                    nidx, nidx, L * D,
                )

                # supertiles within the slab
                lc = 0
                while lc < nch_slab:
                    nch = min(NCHL, nch_slab - lc)
                    ne = nch * 128
                    # transpose gathered [e,d] fp16 tiles -> [d,e] f32 in PSUM
                    psT = ppool.tile([128, 2 * ne], f16, tag="psT")
                    pdT = ppool.tile([128, 2 * ne], f16, tag="pdT")
                    for l in range(L):
                        for cc in range(nch):
                            nc.tensor.transpose(
                                psT[:, l * ne + cc * 128 : l * ne + (cc + 1) * 128],
                                gsrc[:, lc + cc, l * D : (l + 1) * D],
                                id_sb[:],
                            )
                            nc.tensor.transpose(
                                pdT[:, l * ne + cc * 128 : l * ne + (cc + 1) * 128],
                                gdst[:, lc + cc, l * D : (l + 1) * D],
                                id_sb[:],
                            )
                    # dst side to SBUF (ACT), src side read from PSUM by DVE
                    sdT = wpool.tile([128, 2 * ne], f16, tag="sdT")
                    nc.scalar.activation(sdT[:], pdT[:], AF.Copy)

                    # cross products: one TT, free dims (i, j, e)
                    cross = wpool.tile([128, 4 * ne], f32r, tag="cross")
                    s_ap = (
                        psT[:]
                        .rearrange("p (i e) -> p i e", i=2)
                        .unsqueeze(2)
                        .broadcast_to((128, 2, 2, ne))
                    )
                    d_ap = (
                        sdT[:]
                        .rearrange("p (j e) -> p j e", j=2)
                        .unsqueeze(1)
                        .broadcast_to((128, 2, 2, ne))
                    )
                    o_ap = cross[:].rearrange("p (i j e) -> p i j e", i=2, j=2)
                    nc.vector.tensor_tensor(o_ap, s_ap, d_ap, ALU.mult)

                    # mm1: x[dout, e] = sum_k W0k @ cross_k   (f32r)
                    px = ppool.tile([128, ne], f32, tag="px")
                    for k in range(4):
                        nc.tensor.matmul(
                            px[:],
                            w0t_sb[:, k * D : (k + 1) * D],
                            cross[:, k * ne : (k + 1) * ne],
                            start=(k == 0),
                            stop=(k == 3),
                        )
                    x_sb = wpool.tile([128, ne], f32r, tag="x")
                    nc.scalar.activation(x_sb[:], px[:], AF.Relu, bias=b0_sb[:])

                    # mmeff: logits[e, c] per chunk, x as stationary
                    ppos = ppool.tile([128, 2 * nch], f32, tag="ppos")
                    for cc in range(nch):
                        nc.tensor.matmul(
                            ppos[:, cc * 2 : (cc + 1) * 2],
                            x_sb[:, cc * 128 : (cc + 1) * 128],
                            wefft_sb[:],
                            start=True,
                            stop=True,
                        )
                    c_glob = ch0 + lc
                    nc.scalar.activation(
                        logits_sb[:, c_glob * 2 : (c_glob + nch) * 2],
                        ppos[:],
                        AF.Copy,
                    )
                    lc += nch

            # ---- gumbel + compare ----
            eps_ap = u_sb[:, CHL * 2 : CHL * 2 + 1]
            t1 = fpool.tile([128, CHL * 2], f32, tag="t1")
            nc.scalar.activation(t1[:], u_sb[:, : CHL * 2], AF.Ln, bias=eps_ap)
            t2 = fpool.tile([128, CHL * 2], f32, tag="t2")
            nc.scalar.activation(t2[:], t1[:], AF.Ln, bias=eps_ap, scale=-1.0)
            # z = logits - t2  (z = logits + g, g = -t2)
            z = fpool.tile([128, CHL * 2], f32, tag="z")
            nc.vector.tensor_tensor(z[:], logits_sb[:], t2[:], ALU.subtract)
            # margin m = z0 - z1
            marg = fpool.tile([128, CHL], f32, tag="marg")
            z3 = z[:].rearrange("p (c k) -> p c k", k=2)
            nc.vector.tensor_tensor(marg[:], z3[:, :, 0], z3[:, :, 1], ALU.subtract)
            # one-hot
            out_sb = fpool.tile([128, CHL * 2], f32, tag="out")
            o3 = out_sb[:].rearrange("p (c k) -> p c k", k=2)
            nc.vector.tensor_scalar(o3[:, :, 0], marg[:], 0.0, None, ALU.is_ge)
            nc.vector.tensor_scalar(o3[:, :, 1], marg[:], 0.0, None, ALU.is_lt)

            # ---- stores ----
            nc.sync.dma_start(
                outd[:, :].rearrange("(p c) k -> p (c k)", p=128), out_sb[:]
            )
            nc.sync.dma_start(margd[:].rearrange("(p c) -> p c", p=128), marg[:])
    nc.finalize()
    return nc


_PROG_CACHE = {}


def _get_prog():
    if "nc" not in _PROG_CACHE:
        _PROG_CACHE["nc"] = build_program()
    return _PROG_CACHE["nc"]


def _wrap_idx(idx_perm):
    """SBUF index layout: position i -> partition i%16 (replicated x8), col i//16."""
    a = idx_perm.astype(np.int16).reshape(-1, 16)  # [cols, 16]
    sb = np.tile(a.T, (8, 1))  # [128, cols]
    return np.ascontiguousarray(sb)


def _tf32_round(a):
    b = np.asarray(a, np.float32).view(np.uint32).astype(np.uint64)
    lsb = (b >> np.uint64(13)) & np.uint64(1)
    b = b + np.uint64((1 << 12) - 1) + lsb
    b &= np.uint64(~((1 << 13) - 1) & 0xFFFFFFFF)
    return b.astype(np.uint32).view(np.float32)


def _host_prep(h, W0, b0, W1, b1, Wf, bf, u, src, dst):
    h16 = np.ascontiguousarray(
        h.transpose(1, 0, 2).reshape(N, L * D).astype(np.float16)
    )
    w0t = _tf32_round(np.ascontiguousarray(
        np.stack([W0[:, k * D : (k + 1) * D].T for k in range(4)], 0)
        .transpose(1, 0, 2)
        .reshape(D, 4 * D)
    ))
    weff = (Wf.astype(np.float64) @ W1.astype(np.float64)).astype(np.float32)
    wefft = _tf32_round(np.ascontiguousarray(weff.T))
    beff = (
        bf.astype(np.float64) + Wf.astype(np.float64) @ b1.astype(np.float64)
    ).astype(np.float32)
    assert np.all(beff == 0.0), "nonzero beff not folded into device program"
    ident = np.eye(D, dtype=np.float16)

    # per-core permuted gather indices and padded u
    p_idx = np.arange(128)
    c_idx = np.arange(CH)
    # gather position q = c*128 + p maps to e_local = p*157 + c
    e_local = (p_idx[None, :] * CH + c_idx[:, None]).reshape(-1)  # [EPAD] q-major
    in_maps = []
    for k in range(NCORES):
        e_glob = k * E_PER + np.minimum(e_local, E_PER - 1)
        isrc = _wrap_idx(src[e_glob])
        idst = _wrap_idx(dst[e_glob])
        u_pad = np.empty((EPAD, 2), np.float32)
        u_pad[:E_PER] = u[k * E_PER : (k + 1) * E_PER]
        u_pad[E_PER:] = 0.5
        u_arr = np.empty((128, CH * 2 + 1), np.float32)
        u_arr[:, : CH * 2] = u_pad.reshape(128, CH * 2)
        u_arr[:, CH * 2] = EPS
        in_maps.append(
            dict(
                h16=h16, w0t=w0t, wefft=wefft, b0d=b0[:, None].astype(np.float32),
                ident=ident, isrc=isrc, idst=idst, u=np.ascontiguousarray(u_arr),
            )
        )
    # fix key name
    for m in in_maps:
        m["ud"] = m.pop("u")
    return in_maps


def _host_refine(out, marg_all, h, W0, b0, W1, b1, Wf, bf, u, src, dst):
    """Recompute edges with small |margin| in f64 (covers fp16/tf32 noise)."""
    flag = np.nonzero(np.abs(marg_all) < TAU)[0]
    if flag.size == 0:
        return out
    s = src[flag].astype(np.int64)
    d = dst[flag].astype(np.int64)
    h64 = h.astype(np.float64)
    sx = h64[:, s]  # [2, M, 128]
    dx = h64[:, d]
    cross = sx[:, None] * dx[None]  # [2,2,M,128]
    x = np.transpose(cross, (2, 0, 1, 3)).reshape(flag.size, 4 * D)
    x = np.maximum(x @ W0.T.astype(np.float64) + b0.astype(np.float64), 0.0)
    pos = x @ W1.T.astype(np.float64) + b1.astype(np.float64)
    logits = pos @ Wf.T.astype(np.float64) + bf.astype(np.float64)
    g = -np.log(-np.log(u[flag].astype(np.float64) + EPS) + EPS)
    z = logits + g
    cls0 = z[:, 0] >= z[:, 1]
    out[flag, 0] = cls0.astype(np.float32)
    out[flag, 1] = (~cls0).astype(np.float32)
    return out


def kernel(h, W0, b0, W1, b1, Wf, bf, u, src, dst):
    h = np.asarray(h, np.float32)
    W0 = np.asarray(W0, np.float32)
    b0 = np.asarray(b0, np.float32)
    W1 = np.asarray(W1, np.float32)
    b1 = np.asarray(b1, np.float32)
    Wf = np.asarray(Wf, np.float32)
    bf = np.asarray(bf, np.float32)
    u = np.asarray(u, np.float32)
    src = np.asarray(src)
    dst = np.asarray(dst)

    nc = _get_prog()
    in_maps = _host_prep(h, W0, b0, W1, b1, Wf, bf, u, src, dst)
    import os as _os
    _kw = {}
    if _os.environ.get("KBENCH_TRACE"):
        _kw = dict(trace=True, tmpdir=_os.environ.get("KBENCH_TMPDIR") or None)
    res = run_bass_kernel_spmd(nc, in_maps, core_ids=list(range(NCORES)), **_kw)
    _PROG_CACHE["last_res"] = res
    outs = res.results

    out = np.empty((E, 2), np.float32)
    marg_all = np.empty(E, np.float64)
    for k in range(NCORES):
        out[k * E_PER : (k + 1) * E_PER] = outs[k]["outd"][:E_PER]
        marg_all[k * E_PER : (k + 1) * E_PER] = outs[k]["margd"][:E_PER]
    out = _host_refine(out, marg_all, h, W0, b0, W1, b1, Wf, bf, u, src, dst)
    return out


# revision 13
# speedup vs baseline: 1.4354x; 1.4354x over previous
"""Trainium2 Bass kernel for nn_AdversMaskEdge (gnn_message_passing).

Computation (per edge e): gather h[l, src[e]], h[l, dst[e]] (l=0,1, D=128);
cross features x = concat_{i,j} (src_i * dst_j)  [512]; x = relu(x @ W0.T + b0);
pos = x @ W1.T + b1; logits = pos @ Wf.T + bf; z = logits + gumbel(u);
output = one_hot(argmax(z), 2)  (straight-through value == y_hard exactly).

v2 strategy (profiled v1 was GPSIMD-descriptor- and PE-transpose-bound):
  - Shard E=160000 edges over 8 cores (20000 each, padded to 20096 = 157*128);
    each core's edges are SORTED BY SRC NODE on the host. Edge (chunk c, lane p)
    holds sorted edge c*128+p.
  - SRC side: 128 consecutive sorted edges span < 128 distinct nodes, so the
    src "gather" is a selection-matmul: out[d,e] = Hwin.T @ S with a host-staged
    128-node window (wind) and a one-hot selection matrix (seld), both fp16.
    Transpose-free, descriptor-free.
  - DST side: SBUF-source transpose-mode dma_gather from an fp16 node table
    resident in SBUF ([d,e] output layout directly; no HBM reads, no PE
    transposes). The only per-edge descriptor cost left (GPSIMD).
  - MLP: mm1 in float32r with host-prerounded W0^T chunks; W1/Wf folded into
    Weff host-side; logits emitted per-chunk in edge-partition layout by using
    x-chunks as the stationary operand.
  - Gumbel + compare in edge-partition layout; margins written out; edges with
    |margin| < TAU (~300 of 160k; rounding noise is ~5e-4) are recomputed in
    f64 on the host.
"""

import numpy as np

import concourse.bass as bass
import concourse.bacc as bacc
import concourse.mybir as mybir
import concourse.tile as tile
from concourse.bass_utils import run_bass_kernel_spmd

# Problem constants (hardcoded per harness contract)
L, N, D, E = 2, 10000, 128, 160000
EPS = 1e-10
NCORES = 8
E_PER = E // NCORES            # 20000
CH = 157                        # chunks of 128 edges per core
EPAD = 128 * CH                 # 20096
NRANK = (N + 127) // 128        # 79 table ranks
SLAB_CH = 4                     # chunks per dst dma_gather (512 idxs; 1024 crashes transpose-mode)
NCH_ST = 2                      # chunks per compute supertile
TAU = 4e-3                      # |margin| refinement threshold

f32 = mybir.dt.float32
f32r = mybir.dt.float32r
f16 = mybir.dt.float16
i16 = mybir.dt.int16
AF = mybir.ActivationFunctionType
ALU = mybir.AluOpType


def build_program(ch=CH, slab_ch=SLAB_CH, nch_st=NCH_ST):
    CHL, SLABL, NCHL = ch, slab_ch, nch_st
    nc = bacc.Bacc(trn_type="TRN2")

    w0t = nc.dram_tensor("w0t", [D, 4 * D], f32r, kind="ExternalInput")
    wefft = nc.dram_tensor("wefft", [D, 2], f32r, kind="ExternalInput")
    b0d = nc.dram_tensor("b0d", [D, 1], f32, kind="ExternalInput")
    tabd = nc.dram_tensor("tabd", [128, NRANK * 2 * D], f16, kind="ExternalInput")
    wind = nc.dram_tensor("wind", [CHL * 128, 2 * D], f16, kind="ExternalInput")
    seld = nc.dram_tensor("seld", [CHL * 128, 128], f16, kind="ExternalInput")
    idst = nc.dram_tensor("idst", [128, CHL * 8], i16, kind="ExternalInput")
    ud = nc.dram_tensor("ud", [128, CHL * 2 + 1], f32, kind="ExternalInput")
    outd = nc.dram_tensor("outd", [128, CHL * 2], f32, kind="ExternalOutput")
    margd = nc.dram_tensor("margd", [128, CHL], f32, kind="ExternalOutput")

    with tile.TileContext(nc) as tc:
        with (
            tc.tile_pool(name="const", bufs=1) as cpool,
            tc.tile_pool(name="gath", bufs=2) as gpool,
            tc.tile_pool(name="work", bufs=2) as wpool,
            tc.tile_pool(name="psT", bufs=2, space="PSUM") as ppool,
            tc.tile_pool(name="fin", bufs=1) as fpool,
        ):
            # ---- preamble loads ----
            w0t_sb = cpool.tile([D, 4 * D], f32r, tag="w0t")
            nc.sync.dma_start(w0t_sb[:], w0t[:, :])
            wefft_sb = cpool.tile([D, 2], f32r, tag="wefft")
            nc.sync.dma_start(wefft_sb[:], wefft[:, :])
            b0_sb = cpool.tile([D, 1], f32, tag="b0")
            nc.sync.dma_start(b0_sb[:], b0d[:, :])
            tab_sb = cpool.tile([128, NRANK * 2 * D], f16, tag="tab")
            nc.sync.dma_start(tab_sb[:], tabd[:, :])
            idst_sb = cpool.tile([128, CHL * 8], i16, tag="idst")
            nc.sync.dma_start(idst_sb[:], idst[:, :])
            u_sb = fpool.tile([128, CHL * 2 + 1], f32, tag="u")
            nc.sync.dma_start(u_sb[:], ud[:, :])

            logits_sb = fpool.tile([128, CHL * 2], f32, tag="logits")

            # ---- main loop: slabs of SLABL chunks ----
            n_slabs = (CHL + SLABL - 1) // SLABL
            for b in range(n_slabs):
                ch0 = b * SLABL
                nch_slab = min(SLABL, CHL - ch0)
                nidx = nch_slab * 128
                # dst gather (SBUF-source, transposed): [128 d, 2 layers, nidx]
                gdst = gpool.tile([128, 2, nidx], f16, tag="gdst")
                nc.gpsimd.dma_gather(
                    gdst[:], tab_sb[:], idst_sb[:, ch0 * 8 : ch0 * 8 + nidx // 16],
                    nidx, nidx, 2 * D,
                    transpose=True,
                    sbuf_tokens_per_rank=128,
                    sbuf_free_dim_per_rank=2 * D * 2,
                )
                # src windows + selections for the slab (rearranged per lane)
                win_sb = gpool.tile([128, nch_slab * 2 * D], f16, tag="win")
                nc.sync.dma_start(
                    win_sb[:].rearrange("p (c d) -> p c d", c=nch_slab),
                    wind[ch0 * 128 : (ch0 + nch_slab) * 128, :].rearrange(
                        "(c p) d -> p c d", p=128
                    ),
                )
                sel_sb = gpool.tile([128, nch_slab * 128], f16, tag="sel")
                nc.sync.dma_start(
                    sel_sb[:].rearrange("p (c e) -> p c e", c=nch_slab),
                    seld[ch0 * 128 : (ch0 + nch_slab) * 128, :].rearrange(
                        "(c p) e -> p c e", p=128
                    ),
                )

                # supertiles within the slab
                lc = 0
                while lc < nch_slab:
                    nch = min(NCHL, nch_slab - lc)
                    ne = nch * 128
                    # src selection matmuls -> psT [128 d, (layer, e)] f32
                    psT = ppool.tile([128, 2 * ne], f32, tag="psT")
                    for cc in range(nch):
                        for l in range(L):
                            nc.tensor.matmul(
                                psT[:, l * ne + cc * 128 : l * ne + (cc + 1) * 128],
                                win_sb[
                                    :,
                                    (lc + cc) * 2 * D + l * D : (lc + cc) * 2 * D
                                    + (l + 1) * D,
                                ],
                                sel_sb[:, (lc + cc) * 128 : (lc + cc + 1) * 128],
                                start=True,
                                stop=True,
                            )

                    # cross products: one TT, free dims (i, j, e)
                    cross = wpool.tile([128, 4 * ne], f32r, tag="cross")
                    s_ap = (
                        psT[:]
                        .rearrange("p (i e) -> p i e", i=2)
                        .unsqueeze(2)
                        .broadcast_to((128, 2, 2, ne))
                    )
                    d_ap = (
                        gdst[:, :, lc * 128 : lc * 128 + ne]
                        .unsqueeze(1)
                        .broadcast_to((128, 2, 2, ne))
                    )
                    o_ap = cross[:].rearrange("p (i j e) -> p i j e", i=2, j=2)
                    nc.vector.tensor_tensor(o_ap, s_ap, d_ap, ALU.mult)

                    # mm1: x[dout, e] = sum_k W0k @ cross_k   (f32r)
                    px = ppool.tile([128, ne], f32, tag="px")
                    for k in range(4):
                        nc.tensor.matmul(
                            px[:],
                            w0t_sb[:, k * D : (k + 1) * D],
                            cross[:, k * ne : (k + 1) * ne],
                            start=(k == 0),
                            stop=(k == 3),
                        )
                    x_sb = wpool.tile([128, ne], f32r, tag="x")
                    nc.scalar.activation(x_sb[:], px[:], AF.Relu, bias=b0_sb[:])

                    # mmeff: logits[e, c] per chunk, x-chunk as stationary
                    ppos = ppool.tile([128, 2 * nch], f32, tag="ppos")
                    for cc in range(nch):
                        nc.tensor.matmul(
                            ppos[:, cc * 2 : (cc + 1) * 2],
                            x_sb[:, cc * 128 : (cc + 1) * 128],
                            wefft_sb[:],
                            start=True,
                            stop=True,
                        )
                    c_glob = ch0 + lc
                    nc.scalar.activation(
                        logits_sb[:, c_glob * 2 : (c_glob + nch) * 2],
                        ppos[:],
                        AF.Copy,
                    )
                    lc += nch

            # ---- gumbel + compare ----
            eps_ap = u_sb[:, CHL * 2 : CHL * 2 + 1]
            t1 = fpool.tile([128, CHL * 2], f32, tag="t1")
            nc.scalar.activation(t1[:], u_sb[:, : CHL * 2], AF.Ln, bias=eps_ap)
            t2 = fpool.tile([128, CHL * 2], f32, tag="t2")
            nc.scalar.activation(t2[:], t1[:], AF.Ln, bias=eps_ap, scale=-1.0)
            # z = logits - t2  (z = logits + g, g = -t2)
            z = fpool.tile([128, CHL * 2], f32, tag="z")
            nc.vector.tensor_tensor(z[:], logits_sb[:], t2[:], ALU.subtract)
            # margin m = z0 - z1
            marg = fpool.tile([128, CHL], f32, tag="marg")
            z3 = z[:].rearrange("p (c k) -> p c k", k=2)
            nc.vector.tensor_tensor(marg[:], z3[:, :, 0], z3[:, :, 1], ALU.subtract)
            # one-hot
            out_sb = fpool.tile([128, CHL * 2], f32, tag="out")
            o3 = out_sb[:].rearrange("p (c k) -> p c k", k=2)
            nc.vector.tensor_scalar(o3[:, :, 0], marg[:], 0.0, None, ALU.is_ge)
            nc.vector.tensor_scalar(o3[:, :, 1], marg[:], 0.0, None, ALU.is_lt)

            # ---- stores ----
            nc.sync.dma_start(outd[:, :], out_sb[:])
            nc.sync.dma_start(margd[:, :], marg[:])
    nc.finalize()
    return nc


_PROG_CACHE = {}


def _get_prog():
    if "nc" not in _PROG_CACHE:
        _PROG_CACHE["nc"] = build_program()
    return _PROG_CACHE["nc"]


def _tf32_round(a):
    b = np.asarray(a, np.float32).view(np.uint32).astype(np.uint64)
    lsb = (b >> np.uint64(13)) & np.uint64(1)
    b = b + np.uint64((1 << 12) - 1) + lsb
    b &= np.uint64(~((1 << 13) - 1) & 0xFFFFFFFF)
    return b.astype(np.uint32).view(np.float32)


def _wrap_idx(idx_perm):
    """SBUF index layout: position i -> partition i%16 (replicated x8), col i//16."""
    a = idx_perm.astype(np.int16).reshape(-1, 16)  # [cols, 16]
    sb = np.tile(a.T, (8, 1))  # [128, cols]
    return np.ascontiguousarray(sb)


def _host_prep(h, W0, b0, W1, b1, Wf, bf, u, src, dst):
    h16 = np.ascontiguousarray(
        h.transpose(1, 0, 2).reshape(N, L * D).astype(np.float16)
    )
    # dst table: node n -> partition n%128, rank n//128
    tab_pad = np.zeros((NRANK * 128, 2 * D), np.float16)
    tab_pad[:N] = h16
    tabd = np.ascontiguousarray(
        tab_pad.reshape(NRANK, 128, 2 * D).transpose(1, 0, 2).reshape(128, -1)
    )
    w0t = _tf32_round(np.ascontiguousarray(
        np.stack([W0[:, k * D : (k + 1) * D].T for k in range(4)], 0)
        .transpose(1, 0, 2)
        .reshape(D, 4 * D)
    ))
    weff = (Wf.astype(np.float64) @ W1.astype(np.float64)).astype(np.float32)
    wefft = _tf32_round(np.ascontiguousarray(weff.T))
    beff = (
        bf.astype(np.float64) + Wf.astype(np.float64) @ b1.astype(np.float64)
    ).astype(np.float32)
    assert np.all(beff == 0.0), "nonzero beff not folded into device program"

    in_maps = []
    perms = []
    for k in range(NCORES):
        s_slice = src[k * E_PER : (k + 1) * E_PER].astype(np.int64)
        d_slice = dst[k * E_PER : (k + 1) * E_PER].astype(np.int64)
        u_slice = u[k * E_PER : (k + 1) * E_PER]
        perm = np.argsort(s_slice, kind="stable")
        perms.append(perm)
        # padded sorted arrays (pad with the last sorted edge)
        sp = np.empty(EPAD, np.int64)
        dp = np.empty(EPAD, np.int64)
        up = np.empty((EPAD, 2), np.float32)
        sp[:E_PER] = s_slice[perm]
        dp[:E_PER] = d_slice[perm]
        up[:E_PER] = u_slice[perm]
        sp[E_PER:] = sp[E_PER - 1]
        dp[E_PER:] = dp[E_PER - 1]
        up[E_PER:] = 0.5

        # windows + one-hot selections per chunk
        n0 = np.minimum(sp[::128], N - 128)  # [CH]
        rel = sp - np.repeat(n0, 128)
        assert rel.min() >= 0 and rel.max() < 128, "src window overflow"
        win_rows = (n0[:, None] + np.arange(128)[None, :]).reshape(-1)
        wind = h16[win_rows]  # [CH*128, 256]
        sel = np.zeros((CH, 128, 128), np.float16)
        sel[np.repeat(np.arange(CH), 128), rel, np.tile(np.arange(128), CH)] = 1.0
        sel = sel.reshape(CH * 128, 128)

        idst_w = _wrap_idx(dp)
        u_arr = np.empty((128, CH * 2 + 1), np.float32)
        # edge (c,p) = sorted index c*128+p -> u_arr[p, 2c+k]
        u_arr[:, : CH * 2] = up.reshape(CH, 128, 2).transpose(1, 0, 2).reshape(128, -1)
        u_arr[:, CH * 2] = EPS

        in_maps.append(
            dict(
                w0t=w0t, wefft=wefft, b0d=b0[:, None].astype(np.float32),
                tabd=tabd, wind=np.ascontiguousarray(wind),
                seld=np.ascontiguousarray(sel), idst=idst_w,
                ud=np.ascontiguousarray(u_arr),
            )
        )
    return in_maps, perms


def _host_refine(out, marg_all, h, W0, b0, W1, b1, Wf, bf, u, src, dst):
    """Recompute edges with small |margin| in f64 (covers fp16/tf32 noise)."""
    flag = np.nonzero(np.abs(marg_all) < TAU)[0]
    if flag.size == 0:
        return out
    s = src[flag].astype(np.int64)
    d = dst[flag].astype(np.int64)
    h64 = h.astype(np.float64)
    sx = h64[:, s]  # [2, M, 128]
    dx = h64[:, d]
    cross = sx[:, None] * dx[None]  # [2,2,M,128]
    x = np.transpose(cross, (2, 0, 1, 3)).reshape(flag.size, 4 * D)
    x = np.maximum(x @ W0.T.astype(np.float64) + b0.astype(np.float64), 0.0)
    pos = x @ W1.T.astype(np.float64) + b1.astype(np.float64)
    logits = pos @ Wf.T.astype(np.float64) + bf.astype(np.float64)
    g = -np.log(-np.log(u[flag].astype(np.float64) + EPS) + EPS)
    z = logits + g
    cls0 = z[:, 0] >= z[:, 1]
    out[flag, 0] = cls0.astype(np.float32)
    out[flag, 1] = (~cls0).astype(np.float32)
    return out


def kernel(h, W0, b0, W1, b1, Wf, bf, u, src, dst):
    h = np.asarray(h, np.float32)
    W0 = np.asarray(W0, np.float32)
    b0 = np.asarray(b0, np.float32)
    W1 = np.asarray(W1, np.float32)
    b1 = np.asarray(b1, np.float32)
    Wf = np.asarray(Wf, np.float32)
    bf = np.asarray(bf, np.float32)
    u = np.asarray(u, np.float32)
    src = np.asarray(src)
    dst = np.asarray(dst)

    nc = _get_prog()
    in_maps, perms = _host_prep(h, W0, b0, W1, b1, Wf, bf, u, src, dst)
    import os as _os
    _kw = {}
    if _os.environ.get("KBENCH_TRACE"):
        _kw = dict(trace=True, tmpdir=_os.environ.get("KBENCH_TMPDIR") or None)
    res = run_bass_kernel_spmd(nc, in_maps, core_ids=list(range(NCORES)), **_kw)
    _PROG_CACHE["last_res"] = res
    outs = res.results

    out = np.empty((E, 2), np.float32)
    marg_all = np.empty(E, np.float64)
    for k in range(NCORES):
        # device layout [p, 2c+k] -> sorted edge c*128+p
        o = outs[k]["outd"].reshape(128, CH, 2).transpose(1, 0, 2).reshape(EPAD, 2)
        m = outs[k]["margd"].reshape(128, CH).T.reshape(EPAD)
        perm = perms[k]
        out[k * E_PER + perm] = o[:E_PER]
        marg_all[k * E_PER + perm] = m[:E_PER]
    out = _host_refine(out, marg_all, h, W0, b0, W1, b1, Wf, bf, u, src, dst)
    return out
